# revision 17
# baseline (speedup 1.0000x reference)
"""Multi-head attention (B=2, S=2048, D=1024, H=16) on 8 Trainium2 cores.

Sharding: core c -> batch b = c // 4, head group g = c % 4 (4 heads each).
Each core computes its 4 heads end-to-end (QKV proj -> attention -> out-proj
partial) and returns a partial [S, D] output; the host sums the 4 partials
per batch and adds the output bias.

Design notes (v2, from HW trace analysis of the f32r baseline):
  * The PE clock ramps (0.65 -> 1.2 -> 2.4 GHz) only under sustained
    uniform full-size matmuls; the baseline's K=64 score / M=65 PV matmuls
    ran at the 1.2 GHz p-state (427ns per 512 cols instead of 213ns).
    -> every matmul here is K=128 x M=128: per-head K tiles are zero-padded
       to 128 rows (the other head's Q rows hit zero weights), V tiles are
       zero-padded to 128 output columns (col 64 = ones for the softmax
       denominator, 65..127 zeros).
  * bf16 operands everywhere (1 cycle/row like f32r, half the DMA and
    LDWEIGHTS traffic); PSUM accumulation stays f32. rel-err budget 2e-2.
  * One fused exp per kt-pair over [128, 2x1024] amortizes the ScalarE
    access-latency overhead; ScalarE does nothing but Exp (one table load).
  * V is projected directly into [keys, hd] layout (lhsT = X^T d-slab,
    rhs = Wv) with a K=1 ones-matmul seeding the bias row, eliminating the
    baseline's 64 PE transposes.
  * Heads processed serially in attention: PSUM = scores [128,2,1024] (4
    banks) + U [128,1024] (2 banks) + proj/outproj aux 2x[128,512] (2
    banks) = exactly 8 banks.
"""

import numpy as np
import ml_dtypes
from contextlib import ExitStack

import concourse.bass as bass
import concourse.mybir as mybir
import concourse.tile as tile
from concourse import bacc
from concourse.bass import ts, ds
from concourse.bass_utils import run_bass_kernel_spmd
from concourse.masks import make_identity

F32 = mybir.dt.float32
BF16 = mybir.dt.bfloat16
BF16_NP = ml_dtypes.bfloat16

B, S, D = 2, 2048, 1024
H_TOT, HD = 16, 64
HC = 4                 # heads per core
DC = HC * HD           # 256 columns of QKV proj per core
NCORES = 8
P = 128
NDT = D // P           # 8 d-model tiles
NKT = S // P           # 16 key tiles
NQT = S // P           # 16 query tiles
CG = 1024              # q chunk width in attention
NCG = S // CG
SCALE = 1.0 / np.sqrt(HD)
NMM = 512              # matmul moving width (a matmul may not cross a PSUM bank)


def _body(ctx, tc, xq, xk, xv, wq, wk, wv, bq, bk, bv, wo, outp):
    nc = tc.nc
    EXP = mybir.ActivationFunctionType.Exp
    # DRAM bounce buffer for broadcasting softmax denominators across
    # partitions (SBUF APs cannot have a stride-0 partition dim; DRAM can)
    rec_dram = nc.dram_tensor("rec_scratch", [8, CG], F32).ap()

    singles = ctx.enter_context(tc.tile_pool(name="singles", bufs=1))
    xpool = ctx.enter_context(tc.tile_pool(name="xpool", bufs=3))
    wpool = ctx.enter_context(tc.tile_pool(name="wpool", bufs=2))
    ppool = ctx.enter_context(tc.tile_pool(name="ppool", bufs=3))
    opool = ctx.enter_context(tc.tile_pool(name="opool", bufs=2))
    psS = ctx.enter_context(tc.tile_pool(name="psS", bufs=1, space="PSUM"))
    psU = ctx.enter_context(tc.tile_pool(name="psU", bufs=1, space="PSUM"))
    psA = ctx.enter_context(tc.tile_pool(name="psA", bufs=2, space="PSUM"))

    # Persistent per-core tensors (partition dim x free dims)
    QT = [singles.tile([P, S], BF16, tag=f"qt{m}", name=f"qt{m}") for m in range(2)]
    KTe = [singles.tile([P, S], BF16, tag=f"kte{h}", name=f"kte{h}") for h in range(HC)]
    Vt = singles.tile([P, NKT, HC, P], BF16, tag="vt")  # [keys, kt, h, 128]
    OT = [singles.tile([P, S], BF16, tag=f"ot{m}", name=f"ot{m}") for m in range(2)]
    VT = [singles.tile([P, S], F32, tag=f"vt{m}", name=f"vtm{m}") for m in range(2)]
    wo_sb = singles.tile([P, 2, D], BF16, tag="wo")
    identity = singles.tile([P, P], F32, tag="ident")
    ones1f = singles.tile([1, 64], F32, tag="ones1f")
    bv_sb = singles.tile([P, 2], F32, tag="bvs")
    bk_sb = singles.tile([P, 2], F32, tag="bks")
    bq_sb = singles.tile([P, 2], F32, tag="bqs")

    nc.sync.dma_start(out=bv_sb, in_=bv.rearrange("(m p) -> p m", p=P))
    make_identity(nc, identity)

    # Zero/one fills: KTe complement halves, Vt denominator/pad columns
    nc.vector.memset(ones1f, 1.0)
    for h in range(HC):
        zo = 64 * (1 - (h % 2))  # head h occupies rows 64*(h%2); zero the rest
        nc.vector.memset(KTe[h][zo : zo + 64, :], 0.0)
    nc.vector.memset(Vt[:, :, :, HD : HD + 1], 1.0)
    nc.vector.memset(Vt[:, :, :, HD + 1 :], 0.0)

    nc.scalar.dma_start(out=wo_sb, in_=wo.rearrange("(k p) d -> p k d", p=P))
    nc.scalar.dma_start(out=bk_sb, in_=bk.rearrange("(m p) -> p m", p=P))
    nc.scalar.dma_start(out=bq_sb, in_=bq.rearrange("(m p) -> p m", p=P))

    # ---- projections (d-major, full-clock N=512 accumulation chains):
    # V first (Vt build via PE transposes overlaps K/Q), then K into
    # zero-padded per-head tiles, then Q. Bias-add evacuations for V/Q run
    # on the otherwise-idle ScalarE (activation Identity with per-partition
    # bias); K's half-height evacuations stay on the DVE. ----
    for x_dram, w_dram, b_sb_, which in (
        (xv, wv, bv_sb, "v"),
        (xk, wk, bk_sb, "k"),
        (xq, wq, bq_sb, "q"),
    ):
        w_sb = wpool.tile([P, NDT, DC], BF16, tag="w", name="w")
        nc.sync.dma_start(out=w_sb, in_=w_dram.rearrange("(t p) c -> p t c", p=P))
        for cg in range(NCG):
            xt = xpool.tile([P, NDT, CG], BF16, tag="xt", name="xt")
            xsrc = x_dram[:, ts(cg, CG)].rearrange("(t p) q -> p t q", p=P)
            for hh in range(2):
                eng = nc.scalar if hh == 0 else nc.sync
                eng.dma_start(out=xt[:, :, ts(hh, 512)], in_=xsrc[:, :, ts(hh, 512)])
            for m in range(2):
                for c2 in range(2):
                    ps = psA.tile([P, 512], F32, tag="aux")
                    for dt in range(NDT):
                        nc.tensor.matmul(
                            ps,
                            lhsT=w_sb[:, dt, ts(m, P)],
                            rhs=xt[:, dt, ds(c2 * 512, 512)],
                            start=(dt == 0),
                            stop=(dt == NDT - 1),
                        )
                    col = cg * CG + c2 * 512
                    if which == "k":
                        nc.vector.tensor_scalar_add(
                            out=KTe[2 * m][0:64, ds(col, 512)],
                            in0=ps[0:64, :],
                            scalar1=b_sb_[0:64, m : m + 1],
                        )
                        nc.vector.tensor_scalar_add(
                            out=KTe[2 * m + 1][64:128, ds(col, 512)],
                            in0=ps[64:128, :],
                            scalar1=b_sb_[64:128, m : m + 1],
                        )
                    else:
                        dest = VT if which == "v" else QT
                        nc.scalar.add(
                            out=dest[m][:, ds(col, 512)],
                            in_=ps,
                            add=b_sb_[:, m : m + 1],
                        )
        if which == "v":
            # V^T -> Vt[keys, kt, h, hd] via PE transposes (full-clock, ~94ns)
            for h in range(HC):
                m, po = divmod(h, 2)
                for g in range(2):
                    tp = psA.tile([P, 512], F32, tag="aux")
                    for j in range(8):
                        kt = g * 8 + j
                        nc.tensor.transpose(
                            tp[:, ts(j, HD)],
                            VT[m][64 * po : 64 * po + 64, ts(kt, P)],
                            identity[64 * po : 64 * po + 64, 64 * po : 64 * po + 64],
                        )
                    nc.vector.tensor_copy(
                        out=Vt[:, ds(g * 8, 8), h, 0:HD],
                        in_=tp.rearrange("p (j q) -> p j q", q=HD),
                    )

    # ---- attention: heads serial; per kt 2 score MMs -> exp -> 2 PV MMs,
    # all K=128/M=128/N=NMM. Score tiles are double-buffered (2 banks each)
    # so scores(kt+1) overlap exp(kt) on the ScalarE -- the exp latency must
    # stay off the PE's critical path. ----
    # Out-projection emission helpers. The per-engine instruction streams
    # execute IN ORDER, so overlap must be arranged at emission time: cg=0's
    # 32 out-proj matmuls are dribbled one-per-kt into cg=1's scalar-bound
    # attention (PE slack ~200ns/kt); cg=1's 8 q-tiles form the tail.
    op_state = {"pss": None}

    def outproj_mm(pi, tail):
        qt, sub = divmod(pi, 4)
        if tail:
            qt += NQT // NCG
        k2, c2 = divmod(sub, 2)
        if sub == 0:
            op_state["pss"] = [
                psA.tile([P, 512], F32, tag="aux", name=f"aux{i}") for i in range(2)
            ]
        nc.tensor.matmul(
            op_state["pss"][c2],
            lhsT=OT[k2][:, ts(qt, P)],
            rhs=wo_sb[:, k2, ds(c2 * 512, 512)],
            start=(k2 == 0),
            stop=(k2 == 1),
        )
        if sub == 3:
            for c2e in range(2):
                ob = opool.tile([P, 512], F32, tag="ob", bufs=4)
                if tail and c2e == 1:
                    # ScalarE is done with Exp at the tail; during attention a
                    # scalar Copy would thrash the Exp act table.
                    nc.scalar.copy(out=ob, in_=op_state["pss"][c2e])
                else:
                    nc.vector.tensor_copy(out=ob, in_=op_state["pss"][c2e])
                q = nc.scalar if (tail and c2e == 1) else nc.sync
                q.dma_start(out=outp[ts(qt, P), ds(c2e * 512, 512)], in_=ob)

    for cg in range(NCG):
        for hp in range(2):
            for hl in range(2):
                h = 2 * hp + hl
                final = cg == NCG - 1 and hp == 1 and hl == 1
                U = psU.tile([P, CG], F32, tag="u")
                for kt in range(NKT):
                    s = psS.tile([P, CG], F32, tag="s", bufs=2)
                    for c2 in range(CG // NMM):
                        nc.tensor.matmul(
                            s[:, ts(c2, NMM)],
                            lhsT=KTe[h][:, ts(kt, P)],
                            rhs=QT[hp][:, ds(cg * CG + c2 * NMM, NMM)],
                            start=True,
                            stop=True,
                        )
                    p = ppool.tile([P, CG], BF16, tag="p")
                    nc.scalar.activation(out=p, in_=s, func=EXP, scale=float(SCALE))
                    for c2 in range(CG // NMM):
                        nc.tensor.matmul(
                            U[:, ts(c2, NMM)],
                            lhsT=Vt[:, kt, h, :],
                            rhs=p[:, ts(c2, NMM)],
                            start=(kt == 0),
                            stop=(kt == NKT - 1),
                        )
                    if cg == 1:
                        slot = (hp * 2 + hl) * NKT + kt
                        if 8 <= slot < 8 + 4 * (NQT // NCG):
                            outproj_mm(slot - 8, tail=False)

                # softmax denominators: U row 64 holds sum(P).
                usb = opool.tile([HD + 1, CG], F32, tag="usb", bufs=3)
                nc.vector.tensor_copy(out=usb, in_=U[0 : HD + 1, :])
                bc_in = None
                if final:
                    # tail fast path: reciprocal on the single denominator row,
                    # broadcast across 64 partitions by a K=1 f32 ones-matmul
                    # into now-free score PSUM (no DRAM round trip).
                    rrow = opool.tile([1, CG], F32, tag="rrow")
                    nc.vector.reciprocal_approx_fast(out=rrow, in_=usb[HD : HD + 1, :])
                    sps = psS.tile([P, CG], F32, tag="s", bufs=2)
                    for c2 in range(2):
                        nc.tensor.matmul(
                            sps[0:64, ds(c2 * 512, 512)],
                            lhsT=ones1f,
                            rhs=rrow[:, ds(c2 * 512, 512)],
                            start=True,
                            stop=True,
                        )
                    bc_in = sps[0:64, :]
                else:
                    # steady state: broadcast via a DRAM bounce (hidden under
                    # the next head's attention; SBUF APs can't be stride-0)
                    idx = (cg * 2 + hp) * 2 + hl
                    nc.sync.dma_start(
                        out=rec_dram[idx : idx + 1, :], in_=usb[HD : HD + 1, :]
                    )
                    bc = opool.tile([64, CG], F32, tag="bc", bufs=2)
                    row = rec_dram[idx, :]
                    bcast = bass.AP(
                        tensor=row.tensor,
                        offset=row.offset,
                        ap=[[0, 64]] + list(row.ap),
                    )
                    nc.gpsimd.dma_start(out=bc, in_=bcast)
                    nc.vector.reciprocal_approx_fast(out=bc, in_=bc)
                    bc_in = bc
                nc.vector.tensor_mul(
                    out=OT[hp][ds(64 * hl, 64), ts(cg, CG)],
                    in0=usb[0:HD, :],
                    in1=bc_in,
                )

    # tail: cg=1's out-projection q-tiles
    for pi in range(4 * (NQT // NCG)):
        outproj_mm(pi, tail=True)


def build_nc():
    nc = bacc.Bacc("TRN2", target_bir_lowering=False, debug=False)
    aps = {}
    for name, shape, dt_ in (
        ("xq", [D, S], BF16),
        ("xk", [D, S], BF16),
        ("xv", [D, S], BF16),
        ("wq", [D, DC], BF16),
        ("wk", [D, DC], BF16),
        ("wv", [D, DC], BF16),
        ("bq", [DC], F32),
        ("bk", [DC], F32),
        ("bv", [DC], F32),
        ("wo", [DC, D], BF16),
    ):
        aps[name] = nc.dram_tensor(name, shape, dt_, kind="ExternalInput").ap()
    aps["outp"] = nc.dram_tensor("out_partial", [S, D], F32, kind="ExternalOutput").ap()

    with tile.TileContext(nc) as tc:
        with ExitStack() as ctx:
            _body(
                ctx,
                tc,
                aps["xq"], aps["xk"], aps["xv"],
                aps["wq"], aps["wk"], aps["wv"],
                aps["bq"], aps["bk"], aps["bv"],
                aps["wo"], aps["outp"],
            )
    nc.compile()
    return nc


def make_in_maps(inputs):
    q = np.asarray(inputs["query"], dtype=np.float32)
    k = np.asarray(inputs.get("key_", inputs.get("key")), dtype=np.float32)
    v = np.asarray(inputs["value"], dtype=np.float32)
    Wq = np.asarray(inputs["Wq"], dtype=np.float32)
    Wk = np.asarray(inputs["Wk"], dtype=np.float32)
    Wv = np.asarray(inputs["Wv"], dtype=np.float32)
    bq = np.asarray(inputs["bq"], dtype=np.float32)
    bk = np.asarray(inputs["bk"], dtype=np.float32)
    bv = np.asarray(inputs["bv"], dtype=np.float32)
    Wo = np.asarray(inputs["Wo"], dtype=np.float32)

    # one host transpose+cast per batch, shared by the 4 cores of that batch
    qT = [np.ascontiguousarray(q[b].T).astype(BF16_NP) for b in range(B)]
    kT = [np.ascontiguousarray(k[b].T).astype(BF16_NP) for b in range(B)]
    vT = [np.ascontiguousarray(v[b].T).astype(BF16_NP) for b in range(B)]

    in_maps = []
    for c in range(NCORES):
        b, g = divmod(c, 4)
        cs = slice(DC * g, DC * (g + 1))
        in_maps.append(
            {
                "xq": qT[b],
                "xk": kT[b],
                "xv": vT[b],
                "wq": np.ascontiguousarray(Wq[:, cs]).astype(BF16_NP),
                "wk": np.ascontiguousarray(Wk[:, cs]).astype(BF16_NP),
                "wv": np.ascontiguousarray(Wv[:, cs]).astype(BF16_NP),
                "bq": np.ascontiguousarray(bq[cs]),
                "bk": np.ascontiguousarray(bk[cs]),
                "bv": np.ascontiguousarray(bv[cs]),
                "wo": np.ascontiguousarray(Wo[cs, :]).astype(BF16_NP),
            }
        )
    return in_maps


_NC_CACHE = {}


def get_nc():
    if "nc" not in _NC_CACHE:
        _NC_CACHE["nc"] = build_nc()
    return _NC_CACHE["nc"]


def kernel(**inputs):
    nc = get_nc()
    in_maps = make_in_maps(inputs)
    res = run_bass_kernel_spmd(nc, in_maps, list(range(NCORES))).results
    bo = np.asarray(inputs["bo"], dtype=np.float32)
    out = np.empty((B, S, D), dtype=np.float32)
    for b in range(B):
        acc = res[4 * b + 0]["out_partial"].astype(np.float32)
        for g in range(1, 4):
            acc = acc + res[4 * b + g]["out_partial"]
        out[b] = acc + bo[None, :]
    return out
